# revision 4
# baseline (speedup 1.0000x reference)
"""Multi-head attention (B=2, S=2048, D=768, H=12) on 8 Trainium2 NeuronCores.

Sharding: core c handles batch b=c//4 and heads 3*(c%4) .. 3*(c%4)+2.
Each core:
  1. Projects Q,K (feature-major, transposed) and V (sequence-major, with an
     appended ones-column for the softmax denominator) for its 3 heads.
  2. Computes scores^T = K @ Q^T per head (contraction over head_dim=64, heads
     paired into PE row-groups), exp on ScalarE (scores are O(1), no max
     subtraction needed), then ctx^T_aug = V_aug^T @ exp(scores^T) which yields
     both the unnormalized context and the softmax denominator in one pass.
  3. Normalizes, writes local ctx^T [192, 2048] to DRAM.
  4. One 8-rank AllGather -> ctx^T for all heads/batches [1536, 2048].
  5. Indirect-gathers its (batch, s_q quarter) slice and computes the output
     projection y^T[:, q*512:(q+1)*512] = Wo^T @ ctx^T + bo.
Host assembles y[b, q*512:(q+1)*512, :] = out_c^T.

All matmul operands are float32r (TF32-like, full PE rate); accumulation fp32.
"""
import sys

if "/opt/trn_rl_repo" not in sys.path:
    sys.path.insert(0, "/opt/trn_rl_repo")

import numpy as np

B, S, D, H = 2, 2048, 768, 12
HD = 64
P = 128
N_CORES = 8
HPC = 3          # heads per core
NQ = 4           # s_q chunks of 512
SK = 16          # s_k chunks of 128
KD = 6           # D chunks of 128
W = 512          # working free-dim chunk

_CACHE = {}


def _install_profile_shim():
    """run_bass_kernel_spmd(trace=True) needs antenv.axon_hooks; provide it."""
    import contextlib
    import ctypes
    import types

    if "antenv.axon_hooks" in sys.modules:
        return
    try:
        lib = ctypes.CDLL("/opt/axon/libaxon_pjrt.so")
    except OSError:
        return
    if not hasattr(lib, "axon_start_nrt_profile"):
        return
    lib.axon_start_nrt_profile.argtypes = [
        ctypes.POINTER(ctypes.c_int64),
        ctypes.c_size_t,
    ]
    lib.axon_start_nrt_profile.restype = ctypes.c_int64
    lib.axon_stop_nrt_profile.argtypes = [ctypes.c_char_p]
    lib.axon_stop_nrt_profile.restype = ctypes.c_int64

    @contextlib.contextmanager
    def _hook(output_dir, device_ids):
        import jax

        jax.devices()
        if device_ids:
            ids = (ctypes.c_int64 * len(device_ids))(*device_ids)
            rc = lib.axon_start_nrt_profile(ids, len(device_ids))
        else:
            rc = lib.axon_start_nrt_profile(None, 0)
        if rc != 0:
            raise RuntimeError(f"axon_start_nrt_profile rc={rc}")
        try:
            yield
        finally:
            n = lib.axon_stop_nrt_profile(str(output_dir).encode())
            if n < 0:
                raise RuntimeError(f"axon_stop_nrt_profile rc={n}")

    mod = types.ModuleType("antenv.axon_hooks")
    mod.get_axon_ntff_profile_hook = lambda: _hook
    mod.set_axon_ntff_profile_hook = lambda h: None
    sys.modules["antenv.axon_hooks"] = mod


def _build():
    import concourse.bass as bass
    from concourse import bacc
    import concourse.tile as tile
    import concourse.mybir as mybir

    f32r = mybir.dt.float32r
    f32 = mybir.dt.float32
    u32 = mybir.dt.uint32
    AF = mybir.ActivationFunctionType
    ALU = mybir.AluOpType

    nc = bacc.Bacc("TRN2", target_bir_lowering=False, debug=False,
                   num_devices=N_CORES)

    xT = nc.dram_tensor("xT", [D, S], f32r, kind="ExternalInput")
    w_qk = nc.dram_tensor("w_qk", [D, 384], f32r, kind="ExternalInput")
    b_qk = nc.dram_tensor("b_qk", [384, 1], f32, kind="ExternalInput")
    w_v = nc.dram_tensor("w_v", [D, 256], f32r, kind="ExternalInput")
    b_v = nc.dram_tensor("b_v", [1, 256], f32, kind="ExternalInput")
    w_o = nc.dram_tensor("w_o", [D, D], f32r, kind="ExternalInput")
    b_o = nc.dram_tensor("b_o", [D, 1], f32, kind="ExternalInput")
    gidx = nc.dram_tensor("gidx", [D, 1], u32, kind="ExternalInput")
    out = nc.dram_tensor("out", [D, W], f32r, kind="ExternalOutput")

    cc_in = nc.dram_tensor("cc_in", [HPC * HD, S], f32r)
    cc_out = nc.dram_tensor("cc_out", [N_CORES * HPC * HD, S], f32r,
                            addr_space="Shared")

    with tile.TileContext(nc) as tc:
        with tc.tile_pool(name="const", bufs=1) as const, \
             tc.tile_pool(name="qkp", bufs=1) as qkp, \
             tc.tile_pool(name="vp", bufs=1) as vp, \
             tc.tile_pool(name="work", bufs=4) as work, \
             tc.tile_pool(name="expp", bufs=4) as expp, \
             tc.tile_pool(name="gat", bufs=1) as gat, \
             tc.tile_pool(name="outp", bufs=3) as outp:

            # ---- constant loads -------------------------------------------
            wqk = []
            xt = []
            for k in range(KD):
                t = const.tile([P, 384], f32r, tag=f"wqk{k}")
                nc.sync.dma_start(out=t, in_=w_qk[k * P:(k + 1) * P, :])
                wqk.append(t)
            for k in range(KD):
                t = const.tile([P, S], f32r, tag=f"xt{k}")
                nc.sync.dma_start(out=t[:, 0:1024],
                                  in_=xT[k * P:(k + 1) * P, 0:1024])
                nc.sync.dma_start(out=t[:, 1024:2048],
                                  in_=xT[k * P:(k + 1) * P, 1024:2048])
                xt.append(t)
            wv = []
            for k in range(KD):
                t = const.tile([P, 256], f32r, tag=f"wv{k}")
                nc.sync.dma_start(out=t, in_=w_v[k * P:(k + 1) * P, :])
                wv.append(t)
            bqk = []
            for m in range(3):
                t = const.tile([P, 1], f32, tag=f"bqk{m}")
                nc.sync.dma_start(out=t, in_=b_qk[m * P:(m + 1) * P, :])
                bqk.append(t)
            bv = const.tile([P, 256], f32, tag="bv")
            bv_bcast = bass.AP(tensor=b_v[:, :].tensor, offset=0,
                               ap=[[0, P], [1, 256]])
            nc.gpsimd.dma_start(out=bv, in_=bv_bcast)
            wo = []
            bo = []
            gix = []
            for k in range(KD):
                t = const.tile([P, D], f32r, tag=f"wo{k}")
                nc.sync.dma_start(out=t, in_=w_o[k * P:(k + 1) * P, :])
                wo.append(t)
                t = const.tile([P, 1], f32, tag=f"bo{k}")
                nc.sync.dma_start(out=t, in_=b_o[k * P:(k + 1) * P, :])
                bo.append(t)
                t = const.tile([P, 1], u32, tag=f"gix{k}")
                nc.sync.dma_start(out=t, in_=gidx[k * P:(k + 1) * P, :])
                gix.append(t)

            # ---- QK projection: qkt[m] [128, 2048], m-chunks of
            # [K_h0|K_h1], [Q_h0|Q_h1], [K_h2|Q_h2] ------------------------
            qkt = [qkp.tile([P, S], f32r, tag=f"qkt{m}", name=f"qkt{m}") for m in range(3)]
            with tc.tile_pool(name="ps_qk", bufs=2, space="PSUM") as pqk:
                for n in range(NQ):
                    for m in range(3):
                        ps = pqk.tile([P, W], f32)
                        for k in range(KD):
                            nc.tensor.matmul(
                                ps,
                                wqk[k][:, m * P:(m + 1) * P],
                                xt[k][:, n * W:(n + 1) * W],
                                start=(k == 0), stop=(k == KD - 1))
                        nc.vector.tensor_scalar_add(
                            qkt[m][:, n * W:(n + 1) * W], ps, bqk[m])

            # Q_h2 copy to a base-0 tile (SBUF->SBUF DMA partition remap)
            q2c = qkp.tile([64, S], f32r, tag="q2c")
            nc.sync.dma_start(out=q2c[:, :], in_=qkt[2][64:128, :])

            # ---- V projection (sequence-major, x^T chunks stationary) ----
            vsb = [vp.tile([P, 256], f32r, tag=f"v{s}", name=f"v{s}") for s in range(SK)]
            with tc.tile_pool(name="ps_v", bufs=2, space="PSUM") as pv:
                for s_ in range(SK):
                    ps = pv.tile([P, 256], f32)
                    for k in range(KD):
                        nc.tensor.matmul(
                            ps,
                            xt[k][:, s_ * P:(s_ + 1) * P],
                            wv[k],
                            start=(k == 0), stop=(k == KD - 1))
                    nc.vector.tensor_tensor(out=vsb[s_], in0=ps, in1=bv,
                                            op=ALU.add)

            # ---- attention -----------------------------------------------
            def normalize(pc, h, nq):
                den = work.tile([1, W], f32, tag="den")
                nc.vector.tensor_copy(den[0:1, :], pc[64:65, :])
                rec = work.tile([1, W], f32, tag="rec")
                nc.vector.reciprocal(rec, den)
                rb = work.tile([64, W], f32, tag="rb")
                nc.gpsimd.partition_broadcast(rb, rec[:1, :])
                ctx = work.tile([64, W], f32r, tag="ctx")
                nc.vector.tensor_tensor(out=ctx, in0=pc[0:64, :], in1=rb,
                                        op=ALU.mult)
                nc.sync.dma_start(
                    out=cc_in[h * HD:(h + 1) * HD, nq * W:(nq + 1) * W],
                    in_=ctx)

            with tc.tile_pool(name="ps_s", bufs=4, space="PSUM") as pss, \
                 tc.tile_pool(name="ps_c", bufs=3, space="PSUM") as psc:
                for nq in range(NQ):
                    # heads 0,1 run as a PE row-group pair
                    pc0 = psc.tile([65, W], f32, tag="pc")
                    pc1 = psc.tile([65, W], f32, tag="pc")
                    for sk in range(SK):
                        ps0 = pss.tile([P, W], f32, tag="ps")
                        nc.tensor.matmul(
                            ps0,
                            qkt[0][0:64, sk * P:(sk + 1) * P],
                            qkt[1][0:64, nq * W:(nq + 1) * W],
                            start=True, stop=True)
                        ps1 = pss.tile([P, W], f32, tag="ps")
                        nc.tensor.matmul(
                            ps1,
                            qkt[0][64:128, sk * P:(sk + 1) * P],
                            qkt[1][64:128, nq * W:(nq + 1) * W],
                            start=True, stop=True)
                        e0 = expp.tile([P, W], f32r, tag="e")
                        nc.scalar.activation(e0, ps0, AF.Exp)
                        e1 = expp.tile([P, W], f32r, tag="e")
                        nc.scalar.activation(e1, ps1, AF.Exp)
                        nc.tensor.matmul(pc0, vsb[sk][:, 0:65], e0,
                                         start=(sk == 0), stop=(sk == SK - 1))
                        nc.tensor.matmul(pc1, vsb[sk][:, 65:130], e1,
                                         start=(sk == 0), stop=(sk == SK - 1))
                    normalize(pc0, 0, nq)
                    normalize(pc1, 1, nq)
                    # head 2 solo
                    pc2 = psc.tile([65, W], f32, tag="pc")
                    for sk in range(SK):
                        ps2 = pss.tile([P, W], f32, tag="ps")
                        nc.tensor.matmul(
                            ps2,
                            qkt[2][0:64, sk * P:(sk + 1) * P],
                            q2c[:, nq * W:(nq + 1) * W],
                            start=True, stop=True)
                        e2 = expp.tile([P, W], f32r, tag="e")
                        nc.scalar.activation(e2, ps2, AF.Exp)
                        nc.tensor.matmul(pc2, vsb[sk][:, 130:195], e2,
                                         start=(sk == 0), stop=(sk == SK - 1))
                    normalize(pc2, 2, nq)

            # ---- AllGather ------------------------------------------------
            nc.gpsimd.collective_compute(
                "AllGather",
                ALU.bypass,
                ins=[cc_in[:, :]],
                outs=[cc_out[:, :]],
                replica_groups=[list(range(N_CORES))],
            )

            # ---- gather + output projection ------------------------------
            ccv = cc_out[:, :].rearrange("a (b c) -> (a b) c", c=W)
            ctxg = []
            for k in range(KD):
                t = gat.tile([P, W], f32r, tag=f"ctxg{k}", name=f"ctxg{k}")
                nc.gpsimd.indirect_dma_start(
                    out=t,
                    out_offset=None,
                    in_=ccv,
                    in_offset=bass.IndirectOffsetOnAxis(ap=gix[k][:, :1],
                                                        axis=0),
                )
                ctxg.append(t)
            with tc.tile_pool(name="ps_y", bufs=2, space="PSUM") as py:
                for m in range(KD):
                    ps = py.tile([P, W], f32)
                    for k in range(KD):
                        nc.tensor.matmul(
                            ps,
                            wo[k][:, m * P:(m + 1) * P],
                            ctxg[k],
                            start=(k == 0), stop=(k == KD - 1))
                    yt = outp.tile([P, W], f32r, tag="yt")
                    nc.vector.tensor_scalar_add(yt, ps, bo[m])
                    nc.sync.dma_start(out=out[m * P:(m + 1) * P, :], in_=yt)

    nc.compile()
    return nc


def _get_nc():
    if "nc" not in _CACHE:
        _install_profile_shim()
        _CACHE["nc"] = _build()
    return _CACHE["nc"]


def _make_in_maps(x, Wq, bq, Wk, bk, Wv, bv, Wo, bo):
    scale = np.float32(1.0 / np.sqrt(HD))
    f = np.float32
    x, Wq, bq, Wk, bk, Wv, bv, Wo, bo = [
        np.asarray(a, dtype=f) for a in (x, Wq, bq, Wk, bk, Wv, bv, Wo, bo)]

    in_maps = []
    for c in range(N_CORES):
        b = c // 4
        hs = (c % 4) * HPC
        q = c % 4
        hh = [hs, hs + 1, hs + 2]

        def wc(Wm, h):
            return Wm[:, h * HD:(h + 1) * HD]

        def bc(bm, h):
            return bm[h * HD:(h + 1) * HD]

        xTb = np.ascontiguousarray(x[b].T)
        w_qk = np.concatenate(
            [wc(Wk, hh[0]), wc(Wk, hh[1]),
             wc(Wq, hh[0]) * scale, wc(Wq, hh[1]) * scale,
             wc(Wk, hh[2]), wc(Wq, hh[2]) * scale], axis=1)
        b_qk = np.concatenate(
            [bc(bk, hh[0]), bc(bk, hh[1]),
             bc(bq, hh[0]) * scale, bc(bq, hh[1]) * scale,
             bc(bk, hh[2]), bc(bq, hh[2]) * scale])[:, None]
        w_v = np.zeros((D, 256), dtype=f)
        b_v = np.zeros((1, 256), dtype=f)
        for i, h in enumerate(hh):
            w_v[:, i * 65:i * 65 + HD] = wc(Wv, h)
            b_v[0, i * 65:i * 65 + HD] = bc(bv, h)
            b_v[0, i * 65 + HD] = 1.0
        i_feat = np.arange(D, dtype=np.uint32)
        g = (4 * b + i_feat // 192) * 768 + (i_feat % 192) * 4 + q
        in_maps.append({
            "xT": np.ascontiguousarray(xTb),
            "w_qk": np.ascontiguousarray(w_qk),
            "b_qk": np.ascontiguousarray(b_qk),
            "w_v": w_v,
            "b_v": b_v,
            "w_o": np.ascontiguousarray(Wo),
            "b_o": np.ascontiguousarray(bo[:, None]),
            "gidx": g.astype(np.uint32)[:, None],
        })
    return in_maps


def kernel(x, Wq, bq, Wk, bk, Wv, bv, Wo, bo, _trace=False):
    from concourse.bass_utils import run_bass_kernel_spmd

    nc = _get_nc()
    in_maps = _make_in_maps(x, Wq, bq, Wk, bk, Wv, bv, Wo, bo)
    res = run_bass_kernel_spmd(nc, in_maps, list(range(N_CORES)),
                               trace=_trace)
    _CACHE["last_results"] = res
    y = np.empty((B, S, D), dtype=np.float32)
    for c in range(N_CORES):
        b = c // 4
        q = c % 4
        y[b, q * W:(q + 1) * W, :] = res.results[c]["out"].T
    return y


# revision 5
# speedup vs baseline: 1.6487x; 1.6487x over previous
"""Multi-head attention (B=2, S=2048, D=768, H=12) on 8 Trainium2 NeuronCores.

Sharding: core c handles batch b=c//4 and heads 3*(c%4) .. 3*(c%4)+2.
Each core:
  1. Projects Q,K (feature-major, transposed) and V (sequence-major, with an
     appended ones-column for the softmax denominator) for its 3 heads.
  2. Computes scores^T = K @ Q^T per head (contraction over head_dim=64, heads
     paired into PE row-groups), exp on ScalarE (scores are O(1), no max
     subtraction needed), then ctx^T_aug = V_aug^T @ exp(scores^T) which yields
     both the unnormalized context and the softmax denominator in one pass.
  3. Normalizes, writes local ctx^T [192, 2048] to DRAM.
  4. One 8-rank AllGather -> ctx^T for all heads/batches [1536, 2048].
  5. Indirect-gathers its (batch, s_q quarter) slice and computes the output
     projection y^T[:, q*512:(q+1)*512] = Wo^T @ ctx^T + bo.
Host assembles y[b, q*512:(q+1)*512, :] = out_c^T.

All matmul operands are float32r (TF32-like, full PE rate); accumulation fp32.
"""
import sys

if "/opt/trn_rl_repo" not in sys.path:
    sys.path.insert(0, "/opt/trn_rl_repo")

import numpy as np

B, S, D, H = 2, 2048, 768, 12
HD = 64
P = 128
N_CORES = 8
HPC = 3          # heads per core
NQ = 4           # s_q chunks of 512
SK = 16          # s_k chunks of 128
KD = 6           # D chunks of 128
W = 512          # working free-dim chunk

_CACHE = {}


def _install_profile_shim():
    """run_bass_kernel_spmd(trace=True) needs antenv.axon_hooks; provide it."""
    import contextlib
    import ctypes
    import types

    if "antenv.axon_hooks" in sys.modules:
        return
    try:
        lib = ctypes.CDLL("/opt/axon/libaxon_pjrt.so")
    except OSError:
        return
    if not hasattr(lib, "axon_start_nrt_profile"):
        return
    lib.axon_start_nrt_profile.argtypes = [
        ctypes.POINTER(ctypes.c_int64),
        ctypes.c_size_t,
    ]
    lib.axon_start_nrt_profile.restype = ctypes.c_int64
    lib.axon_stop_nrt_profile.argtypes = [ctypes.c_char_p]
    lib.axon_stop_nrt_profile.restype = ctypes.c_int64

    @contextlib.contextmanager
    def _hook(output_dir, device_ids):
        import jax

        jax.devices()
        if device_ids:
            ids = (ctypes.c_int64 * len(device_ids))(*device_ids)
            rc = lib.axon_start_nrt_profile(ids, len(device_ids))
        else:
            rc = lib.axon_start_nrt_profile(None, 0)
        if rc != 0:
            raise RuntimeError(f"axon_start_nrt_profile rc={rc}")
        try:
            yield
        finally:
            n = lib.axon_stop_nrt_profile(str(output_dir).encode())
            if n < 0:
                raise RuntimeError(f"axon_stop_nrt_profile rc={n}")

    mod = types.ModuleType("antenv.axon_hooks")
    mod.get_axon_ntff_profile_hook = lambda: _hook
    mod.set_axon_ntff_profile_hook = lambda h: None
    sys.modules["antenv.axon_hooks"] = mod


def _build():
    import concourse.bass as bass
    from concourse import bacc
    import concourse.tile as tile
    import concourse.mybir as mybir

    f32r = mybir.dt.float32r
    f32 = mybir.dt.float32
    u32 = mybir.dt.uint32
    AF = mybir.ActivationFunctionType
    ALU = mybir.AluOpType

    nc = bacc.Bacc("TRN2", target_bir_lowering=False, debug=False,
                   num_devices=N_CORES)

    xT = nc.dram_tensor("xT", [D, S], f32r, kind="ExternalInput")
    w_qk = nc.dram_tensor("w_qk", [D, 384], f32r, kind="ExternalInput")
    b_qk = nc.dram_tensor("b_qk", [384, 1], f32, kind="ExternalInput")
    w_v = nc.dram_tensor("w_v", [D, 256], f32r, kind="ExternalInput")
    b_v = nc.dram_tensor("b_v", [1, 256], f32, kind="ExternalInput")
    w_o = nc.dram_tensor("w_o", [D, D], f32r, kind="ExternalInput")
    b_o = nc.dram_tensor("b_o", [D, 1], f32, kind="ExternalInput")
    gidx = nc.dram_tensor("gidx", [D, 1], u32, kind="ExternalInput")
    out = nc.dram_tensor("out", [D, W], f32r, kind="ExternalOutput")

    cc_in = nc.dram_tensor("cc_in", [NQ, HPC * HD, W], f32r)
    cc_all = nc.dram_tensor("cc_all", [NQ * N_CORES * HPC * HD, W], f32r,
                            addr_space="Shared")

    with tile.TileContext(nc) as tc:
        with tc.tile_pool(name="const", bufs=1) as const, \
             tc.tile_pool(name="qkp", bufs=1) as qkp, \
             tc.tile_pool(name="vp", bufs=1) as vp, \
             tc.tile_pool(name="work", bufs=4) as work, \
             tc.tile_pool(name="expp", bufs=4) as expp, \
             tc.tile_pool(name="gat", bufs=1) as gat, \
             tc.tile_pool(name="outp", bufs=3) as outp:

            # ---- constant loads -------------------------------------------
            wqk = []
            xt = []
            for k in range(KD):
                t = const.tile([P, 384], f32r, tag=f"wqk{k}")
                nc.sync.dma_start(out=t, in_=w_qk[k * P:(k + 1) * P, :])
                wqk.append(t)
            for k in range(KD):
                t = const.tile([P, S], f32r, tag=f"xt{k}")
                nc.sync.dma_start(out=t[:, 0:1024],
                                  in_=xT[k * P:(k + 1) * P, 0:1024])
                nc.sync.dma_start(out=t[:, 1024:2048],
                                  in_=xT[k * P:(k + 1) * P, 1024:2048])
                xt.append(t)
            wv = []
            for k in range(KD):
                t = const.tile([P, 256], f32r, tag=f"wv{k}")
                nc.sync.dma_start(out=t, in_=w_v[k * P:(k + 1) * P, :])
                wv.append(t)
            bqk = []
            for m in range(3):
                t = const.tile([P, 1], f32, tag=f"bqk{m}")
                nc.sync.dma_start(out=t, in_=b_qk[m * P:(m + 1) * P, :])
                bqk.append(t)
            bv = const.tile([P, 256], f32, tag="bv")
            bv_bcast = bass.AP(tensor=b_v[:, :].tensor, offset=0,
                               ap=[[0, P], [1, 256]])
            nc.gpsimd.dma_start(out=bv, in_=bv_bcast)
            wo = []
            bo = []
            gix = []
            for k in range(KD):
                t = const.tile([P, D], f32r, tag=f"wo{k}")
                nc.sync.dma_start(out=t, in_=w_o[k * P:(k + 1) * P, :])
                wo.append(t)
                t = const.tile([P, 1], f32, tag=f"bo{k}")
                nc.sync.dma_start(out=t, in_=b_o[k * P:(k + 1) * P, :])
                bo.append(t)
                t = const.tile([P, 1], u32, tag=f"gix{k}")
                nc.sync.dma_start(out=t, in_=gidx[k * P:(k + 1) * P, :])
                gix.append(t)

            # ---- QK projection: qkt[m] [128, 2048], m-chunks of
            # [K_h0|K_h1], [Q_h0|Q_h1], [K_h2|Q_h2] ------------------------
            qkt = [qkp.tile([P, S], f32r, tag=f"qkt{m}", name=f"qkt{m}") for m in range(3)]
            with tc.tile_pool(name="ps_qk", bufs=2, space="PSUM") as pqk:
                for n in range(NQ):
                    for m in range(3):
                        ps = pqk.tile([P, W], f32)
                        for k in range(KD):
                            nc.tensor.matmul(
                                ps,
                                wqk[k][:, m * P:(m + 1) * P],
                                xt[k][:, n * W:(n + 1) * W],
                                start=(k == 0), stop=(k == KD - 1))
                        nc.vector.tensor_scalar_add(
                            qkt[m][:, n * W:(n + 1) * W], ps, bqk[m])

            # Q_h2 copy to a base-0 tile (SBUF->SBUF DMA partition remap)
            q2c = qkp.tile([64, S], f32r, tag="q2c")
            nc.sync.dma_start(out=q2c[:, :], in_=qkt[2][64:128, :])

            # ---- V projection (sequence-major, x^T chunks stationary) ----
            vsb = [vp.tile([P, 256], f32r, tag=f"v{s}", name=f"v{s}") for s in range(SK)]
            with tc.tile_pool(name="ps_v", bufs=2, space="PSUM") as pv:
                for s_ in range(SK):
                    ps = pv.tile([P, 256], f32)
                    for k in range(KD):
                        nc.tensor.matmul(
                            ps,
                            xt[k][:, s_ * P:(s_ + 1) * P],
                            wv[k],
                            start=(k == 0), stop=(k == KD - 1))
                    nc.vector.tensor_tensor(out=vsb[s_], in0=ps, in1=bv,
                                            op=ALU.add)

            # ---- attention -----------------------------------------------
            # Chunk = one [s_k 128, s_q 512] score block for one head.
            # Groups of 2 chunks share a 2-bank PSUM tile so one ACT exp
            # covers 1024 columns (amortizes the ~352-cycle ACT overhead).
            # Software-pipelined emission: mm_s(g+1) is emitted before
            # mm_c(g) so the PE never stalls behind the ACT.
            def normalize(pc, nq, h):
                rec = work.tile([1, W], f32, tag="rec")
                nc.vector.reciprocal(rec[0:1, :], pc[64:65, :])
                rb = work.tile([64, W], f32, tag="rb")
                nc.gpsimd.partition_broadcast(rb, rec[:1, :])
                ctx = work.tile([64, W], f32r, tag="ctx")
                nc.vector.tensor_tensor(out=ctx, in0=pc[0:64, :], in1=rb,
                                        op=ALU.mult)
                nc.sync.dma_start(
                    out=cc_in[nq, h * HD:(h + 1) * HD, :],
                    in_=ctx)

            # build group list: per nq, pair phase then solo phase
            groups = []
            for nq in range(NQ):
                for sk in range(SK):
                    groups.append({"nq": nq, "chunks": [(0, sk), (1, sk)],
                                   "last": False})
                for sk in range(0, SK, 2):
                    g = {"nq": nq, "chunks": [(2, sk), (2, sk + 1)],
                         "last": sk == SK - 2}
                    groups.append(g)

            pc_tiles = {}
            cnt = {}

            def emit_mm_s(gi, grp):
                nq = grp["nq"]
                eps = psE.tile([P, 2 * W], f32, tag="ea" if gi % 2 == 0
                               else "eb", name=f"eps{gi}")
                for j, (h, sk) in enumerate(grp["chunks"]):
                    if h == 0:
                        lhsT = qkt[0][0:64, sk * P:(sk + 1) * P]
                        rhs = qkt[1][0:64, nq * W:(nq + 1) * W]
                    elif h == 1:
                        lhsT = qkt[0][64:128, sk * P:(sk + 1) * P]
                        rhs = qkt[1][64:128, nq * W:(nq + 1) * W]
                    else:
                        lhsT = qkt[2][0:64, sk * P:(sk + 1) * P]
                        rhs = q2c[:, nq * W:(nq + 1) * W]
                    nc.tensor.matmul(eps[:, j * W:(j + 1) * W], lhsT, rhs,
                                     start=True, stop=True)
                esb = expp.tile([P, 2 * W], f32r, tag="e", name=f"esb{gi}")
                nc.scalar.activation(esb, eps, AF.Exp)
                return esb

            def emit_mm_c(grp, esb):
                nq = grp["nq"]
                for j, (h, sk) in enumerate(grp["chunks"]):
                    key = (nq, h)
                    if key not in pc_tiles:
                        pc_tiles[key] = psC.tile([65, W], f32, tag="pc",
                                                 name=f"pc{nq}_{h}")
                        cnt[key] = 0
                    nc.tensor.matmul(
                        pc_tiles[key],
                        vsb[sk][:, h * 65:h * 65 + 65],
                        esb[:, j * W:(j + 1) * W],
                        start=(cnt[key] == 0), stop=(cnt[key] == SK - 1))
                    cnt[key] += 1
                    if cnt[key] == SK:
                        normalize(pc_tiles[key], nq, h)
                if grp["last"]:
                    # all of nq's context rows written -> fire this
                    # quarter's AllGather
                    nc.gpsimd.collective_compute(
                        "AllGather",
                        ALU.bypass,
                        ins=[cc_in[nq]],
                        outs=[cc_all[nq * (N_CORES * HPC * HD):
                                     (nq + 1) * (N_CORES * HPC * HD), :]],
                        replica_groups=[list(range(N_CORES))],
                    )

            with tc.tile_pool(name="ps_e", bufs=1, space="PSUM") as psE, \
                 tc.tile_pool(name="ps_c", bufs=3, space="PSUM") as psC:
                prev = None
                for gi, grp in enumerate(groups):
                    esb = emit_mm_s(gi, grp)
                    if prev is not None:
                        emit_mm_c(prev[0], prev[1])
                    prev = (grp, esb)
                emit_mm_c(prev[0], prev[1])

            # ---- gather + output projection ------------------------------
            ctxg = []
            for k in range(KD):
                t = gat.tile([P, W], f32r, tag=f"ctxg{k}", name=f"ctxg{k}")
                nc.gpsimd.indirect_dma_start(
                    out=t,
                    out_offset=None,
                    in_=cc_all[:, :],
                    in_offset=bass.IndirectOffsetOnAxis(ap=gix[k][:, :1],
                                                        axis=0),
                )
                ctxg.append(t)
            with tc.tile_pool(name="ps_y", bufs=2, space="PSUM") as py:
                for m in range(KD):
                    ps = py.tile([P, W], f32)
                    for k in range(KD):
                        nc.tensor.matmul(
                            ps,
                            wo[k][:, m * P:(m + 1) * P],
                            ctxg[k],
                            start=(k == 0), stop=(k == KD - 1))
                    yt = outp.tile([P, W], f32r, tag="yt")
                    nc.vector.tensor_scalar_add(yt, ps, bo[m])
                    nc.sync.dma_start(out=out[m * P:(m + 1) * P, :], in_=yt)

    nc.compile()
    return nc


def _get_nc():
    if "nc" not in _CACHE:
        _install_profile_shim()
        _CACHE["nc"] = _build()
    return _CACHE["nc"]


def _make_in_maps(x, Wq, bq, Wk, bk, Wv, bv, Wo, bo):
    scale = np.float32(1.0 / np.sqrt(HD))
    f = np.float32
    x, Wq, bq, Wk, bk, Wv, bv, Wo, bo = [
        np.asarray(a, dtype=f) for a in (x, Wq, bq, Wk, bk, Wv, bv, Wo, bo)]

    in_maps = []
    for c in range(N_CORES):
        b = c // 4
        hs = (c % 4) * HPC
        q = c % 4
        hh = [hs, hs + 1, hs + 2]

        def wc(Wm, h):
            return Wm[:, h * HD:(h + 1) * HD]

        def bc(bm, h):
            return bm[h * HD:(h + 1) * HD]

        xTb = np.ascontiguousarray(x[b].T)
        w_qk = np.concatenate(
            [wc(Wk, hh[0]), wc(Wk, hh[1]),
             wc(Wq, hh[0]) * scale, wc(Wq, hh[1]) * scale,
             wc(Wk, hh[2]), wc(Wq, hh[2]) * scale], axis=1)
        b_qk = np.concatenate(
            [bc(bk, hh[0]), bc(bk, hh[1]),
             bc(bq, hh[0]) * scale, bc(bq, hh[1]) * scale,
             bc(bk, hh[2]), bc(bq, hh[2]) * scale])[:, None]
        w_v = np.zeros((D, 256), dtype=f)
        b_v = np.zeros((1, 256), dtype=f)
        for i, h in enumerate(hh):
            w_v[:, i * 65:i * 65 + HD] = wc(Wv, h)
            b_v[0, i * 65:i * 65 + HD] = bc(bv, h)
            b_v[0, i * 65 + HD] = 1.0
        i_feat = np.arange(D, dtype=np.uint32)
        g = q * 1536 + (4 * b + i_feat // 192) * 192 + (i_feat % 192)
        in_maps.append({
            "xT": np.ascontiguousarray(xTb),
            "w_qk": np.ascontiguousarray(w_qk),
            "b_qk": np.ascontiguousarray(b_qk),
            "w_v": w_v,
            "b_v": b_v,
            "w_o": np.ascontiguousarray(Wo),
            "b_o": np.ascontiguousarray(bo[:, None]),
            "gidx": g.astype(np.uint32)[:, None],
        })
    return in_maps


def kernel(x, Wq, bq, Wk, bk, Wv, bv, Wo, bo, _trace=False):
    from concourse.bass_utils import run_bass_kernel_spmd

    nc = _get_nc()
    in_maps = _make_in_maps(x, Wq, bq, Wk, bk, Wv, bv, Wo, bo)
    res = run_bass_kernel_spmd(nc, in_maps, list(range(N_CORES)),
                               trace=_trace)
    _CACHE["last_results"] = res
    y = np.empty((B, S, D), dtype=np.float32)
    for c in range(N_CORES):
        b = c // 4
        q = c % 4
        y[b, q * W:(q + 1) * W, :] = res.results[c]["out"].T
    return y
